# revision 33
# baseline (speedup 1.0000x reference)
"""GRU decoder kernel for 8 trn2 NeuronCores — batch-data-parallel, no collectives.

Algorithm (derived from the reference GruDecoder):
  x_{t+1} = y_t = h_{t+1} @ W_fc.T + b_fc, so for t>=1 the input-path matmul
  folds into the recurrence:
      gi_t = h_t @ (W_ih @ W_fc).T + (b_ih + W_ih @ b_fc)
  r/z gates use gi+gh, so those rows of the folded matrix and W_hh are summed
  host-side; the n-gate keeps gi_n / gh_n separate (r multiplies only gh_n).

Sharding: pure data-parallel over batch. Core c owns batch rows [32c, 32c+32).
  The T=256 sequential loop runs locally per core with NO collectives (the
  baseline's per-step AllGather cost ~20ms each through the axon relay).

Per-core per-step work:
  gates.T [4x1024, 32] = W_all.T-tiles @ h.T-tiles   (weight-stationary, PE)
  y       [32, 768]    = h.T-tiles.T @ W_fc.T        (batch-stationary, PE)
  elementwise r/z/n/h_new on [128, 256] tiles        (scalar + vector engines)
  y is int8-quantized on device with a per-(t,row) scale (abs-max over the 768
  outputs); scales ride in NSCL extra rows of the int8 output tensor (bitcast).
  Biases are folded into the matmuls via an extra contraction tile whose rhs
  is a ones-row tile ("ones" below).

h lives on-chip as hT [128, 8*32] bf16: partition p, col j*32+b <-> h[b, 128j+p].
The elementwise output lands directly in this layout, so no transposes at all.

Host runtime: the axon tunnel moves ~35MB/s per client stream but scales with
concurrent client processes, so kernel() spawns NW persistent worker processes
(each owning 8/NW cores end-to-end: execute + fetch + dequantize into a shared
memory output). The in-parent single-process path is kept as a fallback and
serves calls until the workers come up.
"""

import os
import struct
import subprocess
import sys
import time
import weakref

sys.path.insert(0, "/opt/trn_rl_repo")

import numpy as np

H = 1024
OUT = 768
B = 256
T = int(os.environ.get("GRU_T", "256"))
NCORES = 8
BL = B // NCORES  # 32 batch rows per core
KH = H // 128  # 8 contraction tiles over hidden
KX = OUT // 128  # 6 contraction tiles over x (=768)
NSCL = (BL * T * 4 + BL * OUT - 1) // (BL * OUT)  # scale rows in the output
NW = int(os.environ.get("GRU_WORKERS", "8"))

_cache = {}

W_SPECS = [  # per-core prepped weight arrays living in the weights shm
    ("wA", (128, KX + 1, 3 * H)),
    ("wB", (128, KH + 1, 3 * H)),
    ("wrec", (128, KH + 1, 4 * H)),
    ("wfc", (128, KH + 1, OUT)),
    ("ones", (128, BL)),
]
XH_SHAPE = (NCORES * 128, KX + KH, BL)  # global [x0T; h0T], bf16


def _build_program():
    import concourse.mybir as mybir
    from concourse import bacc, tile

    dt = mybir.dt
    AF = mybir.ActivationFunctionType

    nc = bacc.Bacc(num_devices=NCORES)

    wA_d = nc.dram_tensor("wA", [128, KX + 1, 3 * H], dt.bfloat16, kind="ExternalInput")
    wB_d = nc.dram_tensor("wB", [128, KH + 1, 3 * H], dt.bfloat16, kind="ExternalInput")
    wrec_d = nc.dram_tensor("wrec", [128, KH + 1, 4 * H], dt.bfloat16, kind="ExternalInput")
    wfc_d = nc.dram_tensor("wfc", [128, KH + 1, OUT], dt.bfloat16, kind="ExternalInput")
    xh0T_d = nc.dram_tensor(
        "xh0T", [128, KX + KH, BL], dt.bfloat16, kind="ExternalInput"
    )
    ones_d = nc.dram_tensor("ones", [128, BL], dt.bfloat16, kind="ExternalInput")
    # rows 0..T-1: int8 y; rows T..T+NSCL-1 hold the f32 scales bitcast to bytes
    out_d = nc.dram_tensor("out", [T + NSCL, BL, OUT], dt.int8, kind="ExternalOutput")

    with tile.TileContext(nc) as tc:
        with (
            tc.tile_pool(name="wp", bufs=1) as wp,
            tc.tile_pool(name="hp", bufs=3) as hp,
            tc.tile_pool(name="ep", bufs=1) as ep,
            tc.tile_pool(name="yp", bufs=2) as yp,
            tc.tile_pool(name="pp", bufs=1, space="PSUM") as pp,
            tc.tile_pool(name="qp", bufs=2, space="PSUM") as qp,
        ):
            wA = wp.tile([128, KX + 1, 3 * H], dt.bfloat16)
            nc.sync.dma_start(wA[:], wA_d[:])
            wB = wp.tile([128, KH + 1, 3 * H], dt.bfloat16)
            nc.sync.dma_start(wB[:], wB_d[:])
            wrec = wp.tile([128, KH + 1, 4 * H], dt.bfloat16)
            nc.sync.dma_start(wrec[:], wrec_d[:])
            wfc = wp.tile([128, KH + 1, OUT], dt.bfloat16)
            nc.sync.dma_start(wfc[:], wfc_d[:])
            ones = wp.tile([128, BL], dt.bfloat16)
            nc.sync.dma_start(ones[:], ones_d[:])
            x0T = wp.tile([128, KX, BL], dt.bfloat16)
            nc.sync.dma_start(x0T[:], xh0T_d[:, 0:KX, :])

            h = hp.tile([128, KH * BL], dt.bfloat16, tag="h")
            nc.sync.dma_start(
                h[:], xh0T_d[:, KX : KX + KH, :].rearrange("p k b -> p (k b)")
            )

            scl_all = wp.tile([BL, T], dt.float32)

            def hblk(ht, k):
                return ht[:, k * BL : (k + 1) * BL]

            def emit_gates_rec(ht):
                """Recurrent-step gates: 4 psum tiles [128, 8*32]."""
                P = {}
                for g in ("r", "z", "ni", "nh"):
                    P[g] = pp.tile([128, KH * BL], dt.float32, tag=f"P{g}", name=f"P{g}")
                for gi, g in enumerate(("r", "z", "ni", "nh")):
                    for j in range(KH):
                        o = P[g][:, j * BL : (j + 1) * BL]
                        m0 = gi * H + j * 128
                        for k in range(KH + 1):
                            nc.tensor.matmul(
                                o,
                                wrec[:, k, m0 : m0 + 128],
                                hblk(ht, k) if k < KH else ones[:],
                                start=(k == 0),
                                stop=(k == KH),
                            )
                return P

            def emit_gates_step0(ht):
                """Step 0: gi from x0 (wA: r,z,ni), gh from h0 (wB: r,z,nh)."""
                P = {}
                for g in ("r", "z", "ni", "nh"):
                    P[g] = pp.tile([128, KH * BL], dt.float32, tag=f"P{g}", name=f"P{g}")
                gidx_A = {"r": 0, "z": 1, "ni": 2}
                gidx_B = {"r": 0, "z": 1, "nh": 2}
                for g in ("r", "z", "ni", "nh"):
                    for j in range(KH):
                        o = P[g][:, j * BL : (j + 1) * BL]
                        started = False
                        if g in gidx_A:
                            m0 = gidx_A[g] * H + j * 128
                            for k in range(KX + 1):
                                nc.tensor.matmul(
                                    o,
                                    wA[:, k, m0 : m0 + 128],
                                    x0T[:, k, :] if k < KX else ones[:],
                                    start=(k == 0),
                                    stop=(k == KX and g == "ni"),
                                )
                            started = True
                        if g in gidx_B:
                            m0 = gidx_B[g] * H + j * 128
                            for k in range(KH + 1):
                                nc.tensor.matmul(
                                    o,
                                    wB[:, k, m0 : m0 + 128],
                                    hblk(ht, k) if k < KH else ones[:],
                                    start=(k == 0 and not started),
                                    stop=(k == KH),
                                )
                return P

            def emit_elem(P, ht):
                r = ep.tile([128, KH * BL], dt.float32, tag="r")
                nc.scalar.activation(r[:], P["r"][:], AF.Sigmoid)
                z = ep.tile([128, KH * BL], dt.float32, tag="z")
                nc.scalar.activation(z[:], P["z"][:], AF.Sigmoid)
                t2 = ep.tile([128, KH * BL], dt.float32, tag="t2")
                nc.vector.tensor_mul(t2[:], P["nh"][:], r[:])
                t3 = ep.tile([128, KH * BL], dt.float32, tag="t3")
                nc.vector.tensor_add(t3[:], t2[:], P["ni"][:])
                n = ep.tile([128, KH * BL], dt.float32, tag="n")
                nc.scalar.activation(n[:], t3[:], AF.Tanh)
                d = ep.tile([128, KH * BL], dt.float32, tag="d")
                nc.vector.tensor_sub(d[:], ht[:], n[:])
                zd = ep.tile([128, KH * BL], dt.float32, tag="zd")
                nc.vector.tensor_mul(zd[:], z[:], d[:])
                h_new = hp.tile([128, KH * BL], dt.bfloat16, tag="h")
                nc.vector.tensor_add(h_new[:], n[:], zd[:])
                return h_new

            def emit_y(ht, t_out):
                """y = f(ht) [32, 768], int8-quantized with a per-row scale,
                -> out_d[t_out]; the scale (row abs-max) lands in scl_all."""
                Pys = []
                for c in range(2):
                    Py = qp.tile([BL, OUT // 2], dt.float32, tag=f"Py{c}", name=f"Py{c}")
                    cc = slice(c * (OUT // 2), (c + 1) * (OUT // 2))
                    for k in range(KH + 1):
                        nc.tensor.matmul(
                            Py[:],
                            hblk(ht, k) if k < KH else ones[:],
                            wfc[:, k, cc],
                            start=(k == 0),
                            stop=(k == KH),
                        )
                    Pys.append(Py)
                m0 = ep.tile([BL, 1], dt.float32, tag="m0")
                nc.vector.tensor_reduce(
                    m0[:], Pys[0][:], mybir.AxisListType.X, mybir.AluOpType.max,
                    apply_absolute_value=True,
                )
                m1 = ep.tile([BL, 1], dt.float32, tag="m1")
                nc.vector.tensor_reduce(
                    m1[:], Pys[1][:], mybir.AxisListType.X, mybir.AluOpType.max,
                    apply_absolute_value=True,
                )
                mm = scl_all[:, t_out : t_out + 1]
                nc.vector.tensor_max(mm, m0[:], m1[:])
                rec = ep.tile([BL, 1], dt.float32, tag="rec")
                nc.vector.reciprocal(rec[:], mm)
                rec2 = ep.tile([BL, 1], dt.float32, tag="rec2")
                nc.vector.tensor_scalar_mul(rec2[:], rec[:], 126.0)
                q = yp.tile([BL, OUT], dt.int8, tag="q")
                for c in range(2):
                    cc = slice(c * (OUT // 2), (c + 1) * (OUT // 2))
                    nc.scalar.activation(
                        q[:, cc], Pys[c][:], AF.Copy, scale=rec2[:]
                    )
                nc.sync.dma_start(out_d[t_out][:], q[:])

            for t in range(T):
                if t == 0:
                    P = emit_gates_step0(h)
                else:
                    P = emit_gates_rec(h)
                    emit_y(h, t - 1)
                h = emit_elem(P, h)
            emit_y(h, T - 1)
            sbytes = scl_all[:].bitcast(dt.int8)  # [BL, 4*T]
            off = 0
            for r2 in range(NSCL):
                w = min(OUT, 4 * T - off)
                nc.sync.dma_start(out_d[T + r2][:, 0:w], sbytes[:, off : off + w])
                off += w

    nc.compile()
    return nc


def _prep_weights(W_ih, W_hh, b_ih, b_hh, W_fc, b_fc):
    """Per-core (replicated) weight arrays in lhsT tile layouts, bf16."""
    from ml_dtypes import bfloat16

    f32 = np.float32
    W_ih = np.asarray(W_ih, f32)
    W_hh = np.asarray(W_hh, f32)
    b_ih = np.asarray(b_ih, f32)
    b_hh = np.asarray(b_hh, f32)
    W_fc = np.asarray(W_fc, f32)
    b_fc = np.asarray(b_fc, f32)

    W_comb = W_ih @ W_fc  # [3H, H]
    b_comb = b_ih + W_ih @ b_fc  # [3H]

    def ktiles(mat_T, nk, m):
        # mat_T: [K, m] -> [128, nk, m]
        return np.ascontiguousarray(
            mat_T.reshape(nk, 128, m).transpose(1, 0, 2)
        )

    def with_bias(tiles, bias_row):
        # tiles [128, nk, m] + bias ktile (row 0 = bias) -> [128, nk+1, m]
        m = tiles.shape[2]
        bt = np.zeros((128, 1, m), f32)
        bt[0, 0, :] = bias_row
        return np.concatenate([tiles, bt], axis=1)

    R, Z, N = slice(0, H), slice(H, 2 * H), slice(2 * H, 3 * H)

    W_rec = np.concatenate(
        [W_comb[R] + W_hh[R], W_comb[Z] + W_hh[Z], W_comb[N], W_hh[N]], axis=0
    )  # [4H, H]
    b_rec = np.concatenate(
        [b_comb[R] + b_hh[R], b_comb[Z] + b_hh[Z], b_comb[N], b_hh[N]]
    )
    wrec = with_bias(ktiles(W_rec.T, KH, 4 * H), b_rec)

    bA = np.concatenate([b_ih[R] + b_hh[R], b_ih[Z] + b_hh[Z], b_ih[N]])
    wA = with_bias(ktiles(np.ascontiguousarray(W_ih.T), KX, 3 * H), bA)

    bB = np.zeros(3 * H, f32)
    bB[2 * H :] = b_hh[N]
    wB = with_bias(ktiles(np.ascontiguousarray(W_hh.T), KH, 3 * H), bB)

    wfc = with_bias(ktiles(np.ascontiguousarray(W_fc.T), KH, OUT), b_fc)

    ones = np.zeros((128, BL), f32)
    ones[0, :] = 1.0

    bf = bfloat16
    return {
        "wA": wA.astype(bf),
        "wB": wB.astype(bf),
        "wrec": wrec.astype(bf),
        "wfc": wfc.astype(bf),
        "ones": ones.astype(bf),
    }


def _prep_percall(src, hidden):
    """Global (concat over cores) [x0T; h0T] in one array, bf16."""
    from ml_dtypes import bfloat16

    f32 = np.float32
    x0 = np.asarray(src[0], f32)  # [B, OUT]
    h0 = np.asarray(hidden[0], f32)  # [B, H]
    xh = np.empty((NCORES, 128, KX + KH, BL), f32)
    # [c, p, k, b] = x0[32c+b, 128k+p] for k<KX, h0[32c+b, 128(k-KX)+p] after
    xh[:, :, :KX, :] = x0.reshape(NCORES, BL, KX, 128).transpose(0, 3, 2, 1)
    xh[:, :, KX:, :] = h0.reshape(NCORES, BL, KH, 128).transpose(0, 3, 2, 1)
    return xh.reshape(XH_SHAPE).astype(bfloat16)


def _get_runner(nc, lo, hi):
    """Cached jit over shard_map of the bass_exec custom call for devices
    [lo, hi). Mirrors concourse.bass2jax.run_bass_via_pjrt's multi-core
    branch, but the jit object is built once (no per-call retrace) and weight
    arrays can stay device-resident between calls (they are not donated)."""
    import jax
    import concourse.mybir as mybir
    from concourse import bass2jax
    from jax.sharding import Mesh, PartitionSpec, NamedSharding
    from jax.experimental.shard_map import shard_map

    bass2jax.install_neuronx_cc_hook()
    assert nc.dbg_addr is None
    partition_name = nc.partition_id_tensor.name if nc.partition_id_tensor else None

    in_names = []
    out_names = []
    out_avals = []
    zero_shapes = []
    for alloc in nc.m.functions[0].allocations:
        if not isinstance(alloc, mybir.MemoryLocationSet):
            continue
        name = alloc.memorylocations[0].name
        if alloc.kind == "ExternalInput":
            if name != partition_name:
                in_names.append(name)
        elif alloc.kind == "ExternalOutput":
            out_names.append(name)
            shape = tuple(alloc.tensor_shape)
            dtype = mybir.dt.np(alloc.dtype)
            out_avals.append(jax.core.ShapedArray(shape, dtype))
            zero_shapes.append((shape, dtype))
    n_params = len(in_names)
    n_outs = len(out_names)
    all_names = in_names + out_names
    if partition_name is not None:
        all_names = all_names + [partition_name]
    donate = tuple(range(n_params, n_params + n_outs))

    def _body(*args):
        operands = list(args)
        if partition_name is not None:
            operands.append(bass2jax.partition_id_tensor())
        outs = bass2jax._bass_exec_p.bind(
            *operands,
            out_avals=tuple(out_avals),
            in_names=tuple(all_names),
            out_names=tuple(out_names),
            lowering_input_output_aliases=(),
            sim_require_finite=True,
            sim_require_nnan=True,
            nc=nc,
        )
        return tuple(outs)

    nd = hi - lo
    devices = jax.devices()[lo:hi]
    mesh = Mesh(np.asarray(devices), ("core",))
    spec = PartitionSpec("core")
    in_specs = (spec,) * (n_params + n_outs)
    out_specs = (spec,) * n_outs
    sharded = jax.jit(
        shard_map(
            _body, mesh=mesh, in_specs=in_specs, out_specs=out_specs, check_rep=False
        ),
        donate_argnums=donate,
        keep_unused=True,
    )
    sharding = NamedSharding(mesh, spec)

    import jax.numpy as jnp

    # Donated zero output buffers are materialized ON DEVICE by this tiny
    # cached jit — uploading host zeros through the axon tunnel is slow.
    zeros_fn = jax.jit(
        lambda: tuple(
            jnp.zeros((nd * s[0],) + tuple(s[1:]), d) for s, d in zero_shapes
        ),
        out_shardings=(sharding,) * n_outs,
    )
    return sharded, in_names, out_names, zeros_fn, sharding


def _run_cores(runner, w_dev, xh_slice, zeros, out_view, lo, hi):
    """Execute cores [lo,hi) and dequantize their batch slice into out_view
    (a [T, B, OUT] f32 array). Returns the prefetched zeros for next call."""
    import jax

    sharded, in_names, out_names, zeros_fn, sharding = runner
    feeds = dict(w_dev)
    feeds["xh0T"] = xh_slice
    if zeros is None:
        zeros = zeros_fn()
    args = [feeds[nm] for nm in in_names] + list(zeros)
    outs = sharded(*args)
    zeros_next = zeros_fn()  # prefetch for the next call (async)

    q = dict(zip(out_names, outs))["out"]  # [(hi-lo)*(T+NSCL), BL, OUT] int8
    q_shards = q.addressable_shards
    q_datas = jax.device_get([sh.data for sh in q_shards])
    rows = T + NSCL
    for sh, data in zip(q_shards, q_datas):
        c = lo + (sh.index[0].start or 0) // rows
        sbytes = np.ascontiguousarray(
            data[T:].transpose(1, 0, 2).reshape(BL, -1)[:, : T * 4]
        )
        scales = sbytes.view(np.float32)  # [BL, T]
        scal = (scales.T * (1.0 / 126.0))[:, :, None]  # [T, BL, 1]
        np.multiply(data[:T], scal, out=out_view[:, c * BL : (c + 1) * BL, :])
    return zeros_next


# ---------------------------------------------------------------------------
# worker process mode
# ---------------------------------------------------------------------------

def _shm_open(name):
    from multiprocessing import shared_memory

    try:
        return shared_memory.SharedMemory(name=name, track=False)
    except TypeError:  # track= needs python>=3.13
        return shared_memory.SharedMemory(name=name)


def _w_arrays(buf):
    """Map W_SPECS onto the weights shm buffer (after the 8-byte version)."""
    from ml_dtypes import bfloat16

    out = {}
    off = 8
    for name, shape in W_SPECS:
        nbytes = int(np.prod(shape)) * 2
        out[name] = np.ndarray(shape, bfloat16, buffer=buf, offset=off)
        off += nbytes
    return out


def _w_shm_size():
    return 8 + sum(int(np.prod(s)) * 2 for _, s in W_SPECS)


def _worker_main(widx, ndev, shm_w_name, shm_in_name):
    from ml_dtypes import bfloat16
    import jax

    def say(msg):
        sys.stdout.write(f"WRK {msg}\n")
        sys.stdout.flush()

    lo, hi = widx * ndev, (widx + 1) * ndev
    shm_w = _shm_open(shm_w_name)
    shm_in = _shm_open(shm_in_name)
    w_arrs = _w_arrays(shm_w.buf)
    xh_all = np.ndarray(XH_SHAPE, bfloat16, buffer=shm_in.buf, offset=0)

    nc = _build_program()
    runner = _get_runner(nc, lo, hi)
    sharded, in_names, out_names, zeros_fn, sharding = runner

    def put_weights():
        dev = {}
        for k in w_arrs:
            v = np.asarray(w_arrs[k])
            if ndev > 1:
                v = np.ascontiguousarray(np.tile(v, (ndev,) + (1,) * (v.ndim - 1)))
            dev[k] = jax.device_put(v, sharding)
        return dev

    wver = struct.unpack("<q", bytes(shm_w.buf[:8]))[0]
    w_dev = put_weights()

    # warmup: trace + compile-cache load + first execution with the real
    # weights and a zero xh input; output is discarded
    warm_out = np.empty((T, B, OUT), np.float32)
    xh0 = np.zeros((ndev * 128, KX + KH, BL), bfloat16)
    zeros = _run_cores(runner, w_dev, xh0, None, warm_out, lo, hi)
    say("READY")

    for line in sys.stdin:
        parts = line.split()
        if not parts:
            continue
        if parts[0] == "QUIT":
            break
        if parts[0] != "RUN":
            continue
        cid, rver, shm_out_name = int(parts[1]), int(parts[2]), parts[3]
        try:
            if rver != wver:
                wver = rver
                w_dev = put_weights()
            xh = np.ascontiguousarray(xh_all[lo * 128 : hi * 128])
            shm_out = _shm_open(shm_out_name)
            out_view = np.ndarray((T, B, OUT), np.float32, buffer=shm_out.buf)
            zeros = _run_cores(runner, w_dev, xh, zeros, out_view, lo, hi)
            del out_view
            shm_out.close()
            say(f"DONE {cid}")
        except Exception as e:  # noqa: BLE001
            say(f"ERR {cid} {type(e).__name__}:{e!r}")
            break


# ---------------------------------------------------------------------------
# parent
# ---------------------------------------------------------------------------

def _start_workers():
    from multiprocessing import shared_memory

    tag = f"{os.getpid()}_{int(time.time()) % 100000}"
    shm_w = shared_memory.SharedMemory(
        name=f"gruw_{tag}", create=True, size=_w_shm_size()
    )
    shm_in = shared_memory.SharedMemory(
        name=f"grui_{tag}", create=True, size=int(np.prod(XH_SHAPE)) * 2
    )
    ndev = NCORES // NW
    procs = []
    for w in range(NW):
        p = subprocess.Popen(
            [
                sys.executable,
                os.path.abspath(__file__),
                "--gru-worker",
                str(w),
                str(ndev),
                shm_w.name,
                shm_in.name,
            ],
            stdin=subprocess.PIPE,
            stdout=subprocess.PIPE,
            stderr=subprocess.DEVNULL,
            text=True,
        )
        procs.append(p)
    _cache["workers"] = {
        "procs": procs,
        "shm_w": shm_w,
        "shm_in": shm_in,
        "ready": [False] * NW,
        "bufs": [bytearray() for _ in range(NW)],
        "lines": [],
        "dead": False,
        "cid": 0,
    }


def _drain_worker_lines(st, timeout_s):
    """Read whatever is available on worker stdouts within timeout_s; append
    decoded protocol lines to st["lines"] as (widx, line). Raises on a dead
    worker pipe."""
    import select

    fdmap = {p.stdout.fileno(): w for w, p in enumerate(st["procs"])}
    r, _, _ = select.select(list(fdmap), [], [], max(0.0, timeout_s))
    for fd in r:
        data = os.read(fd, 65536)
        if not data:
            raise RuntimeError("worker eof")
        w = fdmap[fd]
        st["bufs"][w] += data
        while b"\n" in st["bufs"][w]:
            line, _, st["bufs"][w] = st["bufs"][w].partition(b"\n")
            line = line.strip()
            if line.startswith(b"WRK "):
                st["lines"].append((w, line.decode(errors="replace")))


def _workers_poll_ready(block_s=0.0):
    """Check (waiting at most block_s) that all workers have printed READY."""
    st = _cache.get("workers")
    if st is None or st["dead"] or not st["procs"]:
        return False
    deadline = time.time() + block_s
    while True:
        for w, line in st["lines"]:
            if line == "WRK READY":
                st["ready"][w] = True
        st["lines"] = [
            (w, l) for w, l in st["lines"] if l != "WRK READY"
        ]
        if all(st["ready"]):
            return True
        for p in st["procs"]:
            if p.poll() is not None:
                st["dead"] = True
                return False
        left = deadline - time.time()
        if left <= 0:
            return False
        try:
            _drain_worker_lines(st, left)
        except RuntimeError:
            st["dead"] = True
            return False


def _workers_run(xh0T, wver, timeout_s=180.0):
    """Dispatch one call to the workers; returns the shm-backed output array
    or None on failure."""
    from multiprocessing import shared_memory

    st = _cache["workers"]
    xh_view = np.ndarray(XH_SHAPE, xh0T.dtype, buffer=st["shm_in"].buf)
    xh_view[:] = xh0T
    st["cid"] += 1
    cid = st["cid"]
    shm_out = shared_memory.SharedMemory(
        name=f"gruo_{os.getpid()}_{cid}", create=True, size=T * B * OUT * 4
    )
    try:
        for p in st["procs"]:
            p.stdin.write(f"RUN {cid} {wver} {shm_out.name}\n")
            p.stdin.flush()
        done = set()
        deadline = time.time() + timeout_s
        while len(done) < NW:
            for w, line in st["lines"]:
                if line == f"WRK DONE {cid}":
                    done.add(w)
                elif line.startswith("WRK ERR"):
                    raise RuntimeError(line)
            st["lines"] = []
            if len(done) >= NW:
                break
            left = deadline - time.time()
            if left <= 0:
                raise TimeoutError("worker deadline")
            _drain_worker_lines(st, left)
    except Exception:
        st["dead"] = True
        shm_out.close()
        try:
            shm_out.unlink()
        except FileNotFoundError:
            pass
        return None
    arr = np.ndarray((T, B, OUT), np.float32, buffer=shm_out.buf)

    def _cleanup(s=shm_out):
        s.close()
        try:
            s.unlink()
        except FileNotFoundError:
            pass

    weakref.finalize(arr, _cleanup)
    return arr


def _weights_fingerprint_ok(args):
    saved = _cache.get("w_args")
    if saved is None:
        return False
    return all(np.array_equal(a, b) for a, b in zip(saved, args))


def kernel(src, tgt, hidden, W_ih, W_hh, b_ih, b_hh, W_fc, b_fc, **_unused):
    import jax

    if "nc" not in _cache:
        _cache["nc"] = _build_program()
        _cache["runner"] = _get_runner(_cache["nc"], 0, NCORES)
        _cache["wver"] = 0

    w_args = (W_ih, W_hh, b_ih, b_hh, W_fc, b_fc)
    if not _weights_fingerprint_ok(w_args):
        w = _prep_weights(*w_args)
        sharding = _cache["runner"][4]
        dev = {}
        for k, v in w.items():
            g = np.ascontiguousarray(np.tile(v, (NCORES,) + (1,) * (v.ndim - 1)))
            dev[k] = jax.device_put(g, sharding)
        _cache["w_dev"] = dev
        _cache["w_host"] = w
        _cache["w_args"] = tuple(np.asarray(a) for a in w_args)
        _cache["wver"] += 1
        st = _cache.get("workers")
        if st is not None and not st["dead"]:
            wv = _w_arrays(st["shm_w"].buf)
            for k, v in w.items():
                wv[k][:] = v
            st["shm_w"].buf[:8] = struct.pack("<q", _cache["wver"])

    if NW > 0 and "workers" not in _cache:
        try:
            _start_workers()
            st = _cache["workers"]
            wv = _w_arrays(st["shm_w"].buf)
            for k, v in _cache["w_host"].items():
                wv[k][:] = v
            st["shm_w"].buf[:8] = struct.pack("<q", _cache["wver"])
        except Exception:
            _cache["workers"] = {"dead": True, "ready": [], "procs": []}

    xh0T = _prep_percall(src, hidden)

    if NW > 0 and _workers_poll_ready(0.05):
        out = _workers_run(xh0T, _cache["wver"])
        if out is not None:
            return out

    # fallback / warm-up path: single-process run over all 8 cores
    full = np.empty((T, B, OUT), np.float32)
    _cache["zeros_next"] = _run_cores(
        _cache["runner"],
        _cache["w_dev"],
        xh0T,
        _cache.pop("zeros_next", None),
        full,
        0,
        NCORES,
    )
    return full


if __name__ == "__main__":
    if len(sys.argv) >= 2 and sys.argv[1] == "--gru-worker":
        _worker_main(
            int(sys.argv[2]), int(sys.argv[3]), sys.argv[4], sys.argv[5]
        )


# revision 34
# speedup vs baseline: 1.0494x; 1.0494x over previous
"""GRU decoder kernel for 8 trn2 NeuronCores — batch-data-parallel, no collectives.

Algorithm (derived from the reference GruDecoder):
  x_{t+1} = y_t = h_{t+1} @ W_fc.T + b_fc, so for t>=1 the input-path matmul
  folds into the recurrence:
      gi_t = h_t @ (W_ih @ W_fc).T + (b_ih + W_ih @ b_fc)
  r/z gates use gi+gh, so those rows of the folded matrix and W_hh are summed
  host-side; the n-gate keeps gi_n / gh_n separate (r multiplies only gh_n).

Sharding: pure data-parallel over batch. Core c owns batch rows [32c, 32c+32).
  The T=256 sequential loop runs locally per core with NO collectives (the
  baseline's per-step AllGather cost ~20ms each through the axon relay).

Per-core per-step work:
  gates.T [4x1024, 32] = W_all.T-tiles @ h.T-tiles   (weight-stationary, PE)
  y.T     [32, 768]    = h.T-tiles.T @ W_fc.T        (batch-stationary, PE)
  elementwise r/z/n/h_new on [128, 256] tiles        (scalar + vector engines)
  Biases are folded into the matmuls via an extra contraction tile whose rhs
  is a ones-row tile ("ones" below).

h lives on-chip as hT [128, 8*32] bf16: partition p, col j*32+b <-> h[b, 128j+p].
The elementwise output lands directly in this layout, so no transposes at all.
y is produced batch-major [32, 768] so the host only concatenates batch slices.
"""

import os
import sys

sys.path.insert(0, "/opt/trn_rl_repo")

import numpy as np

H = 1024
OUT = 768
B = 256
T = int(os.environ.get("GRU_T", "256"))
NCORES = 8
BL = B // NCORES  # 32 batch rows per core
KH = H // 128  # 8 contraction tiles over hidden
KX = OUT // 128  # 6 contraction tiles over x (=768)

_cache = {}


def _build_program():
    import concourse.mybir as mybir
    from concourse import bacc, tile

    dt = mybir.dt
    AF = mybir.ActivationFunctionType

    nc = bacc.Bacc(num_devices=NCORES)

    wA_d = nc.dram_tensor("wA", [128, KX + 1, 3 * H], dt.bfloat16, kind="ExternalInput")
    wB_d = nc.dram_tensor("wB", [128, KH + 1, 3 * H], dt.bfloat16, kind="ExternalInput")
    wrec_d = nc.dram_tensor("wrec", [128, KH + 1, 4 * H], dt.bfloat16, kind="ExternalInput")
    wfc_d = nc.dram_tensor("wfc", [128, KH + 1, OUT], dt.bfloat16, kind="ExternalInput")
    xh0T_d = nc.dram_tensor(
        "xh0T", [128, KX + KH, BL], dt.bfloat16, kind="ExternalInput"
    )
    ones_d = nc.dram_tensor("ones", [128, BL], dt.bfloat16, kind="ExternalInput")
    # rows 0..T-1: int8 y; rows T..T+NSCL-1 hold the f32 scales bitcast to bytes
    NSCL = (BL * T * 4 + BL * OUT - 1) // (BL * OUT)
    out_d = nc.dram_tensor("out", [T + NSCL, BL, OUT], dt.int8, kind="ExternalOutput")

    with tile.TileContext(nc) as tc:
        with (
            tc.tile_pool(name="wp", bufs=1) as wp,
            tc.tile_pool(name="hp", bufs=3) as hp,
            tc.tile_pool(name="ep", bufs=1) as ep,
            tc.tile_pool(name="yp", bufs=2) as yp,
            tc.tile_pool(name="pp", bufs=1, space="PSUM") as pp,
            tc.tile_pool(name="qp", bufs=2, space="PSUM") as qp,
        ):
            wA = wp.tile([128, KX + 1, 3 * H], dt.bfloat16)
            nc.sync.dma_start(wA[:], wA_d[:])
            wB = wp.tile([128, KH + 1, 3 * H], dt.bfloat16)
            nc.sync.dma_start(wB[:], wB_d[:])
            wrec = wp.tile([128, KH + 1, 4 * H], dt.bfloat16)
            nc.sync.dma_start(wrec[:], wrec_d[:])
            wfc = wp.tile([128, KH + 1, OUT], dt.bfloat16)
            nc.sync.dma_start(wfc[:], wfc_d[:])
            ones = wp.tile([128, BL], dt.bfloat16)
            nc.sync.dma_start(ones[:], ones_d[:])
            x0T = wp.tile([128, KX, BL], dt.bfloat16)
            nc.sync.dma_start(x0T[:], xh0T_d[:, 0:KX, :])

            h = hp.tile([128, KH * BL], dt.bfloat16, tag="h")
            nc.sync.dma_start(
                h[:], xh0T_d[:, KX : KX + KH, :].rearrange("p k b -> p (k b)")
            )

            scl_all = wp.tile([BL, T], dt.float32)

            def hblk(ht, k):
                return ht[:, k * BL : (k + 1) * BL]

            def emit_gates_rec(ht):
                """Recurrent-step gates: 4 psum tiles [128, 8*32]."""
                P = {}
                for g in ("r", "z", "ni", "nh"):
                    P[g] = pp.tile([128, KH * BL], dt.float32, tag=f"P{g}", name=f"P{g}")
                for gi, g in enumerate(("r", "z", "ni", "nh")):
                    for j in range(KH):
                        o = P[g][:, j * BL : (j + 1) * BL]
                        m0 = gi * H + j * 128
                        for k in range(KH + 1):
                            nc.tensor.matmul(
                                o,
                                wrec[:, k, m0 : m0 + 128],
                                hblk(ht, k) if k < KH else ones[:],
                                start=(k == 0),
                                stop=(k == KH),
                            )
                return P

            def emit_gates_step0(ht):
                """Step 0: gi from x0 (wA: r,z,ni), gh from h0 (wB: r,z,nh)."""
                P = {}
                for g in ("r", "z", "ni", "nh"):
                    P[g] = pp.tile([128, KH * BL], dt.float32, tag=f"P{g}", name=f"P{g}")
                gidx_A = {"r": 0, "z": 1, "ni": 2}
                gidx_B = {"r": 0, "z": 1, "nh": 2}
                for g in ("r", "z", "ni", "nh"):
                    for j in range(KH):
                        o = P[g][:, j * BL : (j + 1) * BL]
                        started = False
                        if g in gidx_A:
                            m0 = gidx_A[g] * H + j * 128
                            for k in range(KX + 1):
                                nc.tensor.matmul(
                                    o,
                                    wA[:, k, m0 : m0 + 128],
                                    x0T[:, k, :] if k < KX else ones[:],
                                    start=(k == 0),
                                    stop=(k == KX and g == "ni"),
                                )
                            started = True
                        if g in gidx_B:
                            m0 = gidx_B[g] * H + j * 128
                            for k in range(KH + 1):
                                nc.tensor.matmul(
                                    o,
                                    wB[:, k, m0 : m0 + 128],
                                    hblk(ht, k) if k < KH else ones[:],
                                    start=(k == 0 and not started),
                                    stop=(k == KH),
                                )
                return P

            def emit_elem(P, ht):
                r = ep.tile([128, KH * BL], dt.float32, tag="r")
                nc.scalar.activation(r[:], P["r"][:], AF.Sigmoid)
                z = ep.tile([128, KH * BL], dt.float32, tag="z")
                nc.scalar.activation(z[:], P["z"][:], AF.Sigmoid)
                t2 = ep.tile([128, KH * BL], dt.float32, tag="t2")
                nc.vector.tensor_mul(t2[:], P["nh"][:], r[:])
                t3 = ep.tile([128, KH * BL], dt.float32, tag="t3")
                nc.vector.tensor_add(t3[:], t2[:], P["ni"][:])
                n = ep.tile([128, KH * BL], dt.float32, tag="n")
                nc.scalar.activation(n[:], t3[:], AF.Tanh)
                d = ep.tile([128, KH * BL], dt.float32, tag="d")
                nc.vector.tensor_sub(d[:], ht[:], n[:])
                zd = ep.tile([128, KH * BL], dt.float32, tag="zd")
                nc.vector.tensor_mul(zd[:], z[:], d[:])
                h_new = hp.tile([128, KH * BL], dt.bfloat16, tag="h")
                nc.vector.tensor_add(h_new[:], n[:], zd[:])
                return h_new

            def emit_y(ht, t_out):
                """y = f(ht) [32, 768], int8-quantized with a per-row scale,
                -> out_d[t_out]; the scale (row abs-max) lands in scl_all."""
                Pys = []
                for c in range(2):
                    Py = qp.tile([BL, OUT // 2], dt.float32, tag=f"Py{c}", name=f"Py{c}")
                    cc = slice(c * (OUT // 2), (c + 1) * (OUT // 2))
                    for k in range(KH + 1):
                        nc.tensor.matmul(
                            Py[:],
                            hblk(ht, k) if k < KH else ones[:],
                            wfc[:, k, cc],
                            start=(k == 0),
                            stop=(k == KH),
                        )
                    Pys.append(Py)
                m0 = ep.tile([BL, 1], dt.float32, tag="m0")
                nc.vector.tensor_reduce(
                    m0[:], Pys[0][:], mybir.AxisListType.X, mybir.AluOpType.max,
                    apply_absolute_value=True,
                )
                m1 = ep.tile([BL, 1], dt.float32, tag="m1")
                nc.vector.tensor_reduce(
                    m1[:], Pys[1][:], mybir.AxisListType.X, mybir.AluOpType.max,
                    apply_absolute_value=True,
                )
                mm = scl_all[:, t_out : t_out + 1]
                nc.vector.tensor_max(mm, m0[:], m1[:])
                rec = ep.tile([BL, 1], dt.float32, tag="rec")
                nc.vector.reciprocal(rec[:], mm)
                rec2 = ep.tile([BL, 1], dt.float32, tag="rec2")
                nc.vector.tensor_scalar_mul(rec2[:], rec[:], 126.0)
                q = yp.tile([BL, OUT], dt.int8, tag="q")
                for c in range(2):
                    cc = slice(c * (OUT // 2), (c + 1) * (OUT // 2))
                    nc.scalar.activation(
                        q[:, cc], Pys[c][:], AF.Copy, scale=rec2[:]
                    )
                nc.sync.dma_start(out_d[t_out][:], q[:])

            for t in range(T):
                if t == 0:
                    P = emit_gates_step0(h)
                else:
                    P = emit_gates_rec(h)
                    emit_y(h, t - 1)
                h = emit_elem(P, h)
            emit_y(h, T - 1)
            sbytes = scl_all[:].bitcast(dt.int8)  # [BL, 4*T]
            off = 0
            for r2 in range(NSCL):
                w = min(OUT, 4 * T - off)
                nc.sync.dma_start(out_d[T + r2][:, 0:w], sbytes[:, off : off + w])
                off += w

    nc.compile()
    return nc


def _prep_weights(W_ih, W_hh, b_ih, b_hh, W_fc, b_fc):
    """Per-core (replicated) weight arrays in lhsT tile layouts, bf16."""
    from ml_dtypes import bfloat16

    f32 = np.float32
    W_ih = np.asarray(W_ih, f32)
    W_hh = np.asarray(W_hh, f32)
    b_ih = np.asarray(b_ih, f32)
    b_hh = np.asarray(b_hh, f32)
    W_fc = np.asarray(W_fc, f32)
    b_fc = np.asarray(b_fc, f32)

    W_comb = W_ih @ W_fc  # [3H, H]
    b_comb = b_ih + W_ih @ b_fc  # [3H]

    def ktiles(mat_T, nk, m):
        # mat_T: [K, m] -> [128, nk, m]
        return np.ascontiguousarray(
            mat_T.reshape(nk, 128, m).transpose(1, 0, 2)
        )

    def with_bias(tiles, bias_row):
        # tiles [128, nk, m] + bias ktile (row 0 = bias) -> [128, nk+1, m]
        m = tiles.shape[2]
        bt = np.zeros((128, 1, m), f32)
        bt[0, 0, :] = bias_row
        return np.concatenate([tiles, bt], axis=1)

    R, Z, N = slice(0, H), slice(H, 2 * H), slice(2 * H, 3 * H)

    W_rec = np.concatenate(
        [W_comb[R] + W_hh[R], W_comb[Z] + W_hh[Z], W_comb[N], W_hh[N]], axis=0
    )  # [4H, H]
    b_rec = np.concatenate(
        [b_comb[R] + b_hh[R], b_comb[Z] + b_hh[Z], b_comb[N], b_hh[N]]
    )
    wrec = with_bias(ktiles(W_rec.T, KH, 4 * H), b_rec)

    bA = np.concatenate([b_ih[R] + b_hh[R], b_ih[Z] + b_hh[Z], b_ih[N]])
    wA = with_bias(ktiles(np.ascontiguousarray(W_ih.T), KX, 3 * H), bA)

    bB = np.zeros(3 * H, f32)
    bB[2 * H :] = b_hh[N]
    wB = with_bias(ktiles(np.ascontiguousarray(W_hh.T), KH, 3 * H), bB)

    wfc = with_bias(ktiles(np.ascontiguousarray(W_fc.T), KH, OUT), b_fc)

    ones = np.zeros((128, BL), f32)
    ones[0, :] = 1.0

    bf = bfloat16
    return {
        "wA": wA.astype(bf),
        "wB": wB.astype(bf),
        "wrec": wrec.astype(bf),
        "wfc": wfc.astype(bf),
        "ones": ones.astype(bf),
    }


def _prep_percall(src, hidden):
    """Global (concat over cores) [x0T; h0T] in one array, bf16."""
    from ml_dtypes import bfloat16

    f32 = np.float32
    x0 = np.asarray(src[0], f32)  # [B, OUT]
    h0 = np.asarray(hidden[0], f32)  # [B, H]
    xh = np.empty((NCORES, 128, KX + KH, BL), f32)
    # [c, p, k, b] = x0[32c+b, 128k+p] for k<KX, h0[32c+b, 128(k-KX)+p] after
    xh[:, :, :KX, :] = x0.reshape(NCORES, BL, KX, 128).transpose(0, 3, 2, 1)
    xh[:, :, KX:, :] = h0.reshape(NCORES, BL, KH, 128).transpose(0, 3, 2, 1)
    return xh.reshape(NCORES * 128, KX + KH, BL).astype(bfloat16)


def _get_runner(nc):
    """Cached jit over shard_map of the bass_exec custom call.

    Mirrors concourse.bass2jax.run_bass_via_pjrt's multi-core branch, but the
    jit object is built once so later calls skip retracing, and weight arrays
    can stay device-resident between calls (they are not donated).
    """
    import jax
    import concourse.mybir as mybir
    from concourse import bass2jax
    from jax.sharding import Mesh, PartitionSpec, NamedSharding
    from jax.experimental.shard_map import shard_map

    bass2jax.install_neuronx_cc_hook()
    assert nc.dbg_addr is None
    partition_name = nc.partition_id_tensor.name if nc.partition_id_tensor else None

    in_names = []
    out_names = []
    out_avals = []
    zero_shapes = []
    for alloc in nc.m.functions[0].allocations:
        if not isinstance(alloc, mybir.MemoryLocationSet):
            continue
        name = alloc.memorylocations[0].name
        if alloc.kind == "ExternalInput":
            if name != partition_name:
                in_names.append(name)
        elif alloc.kind == "ExternalOutput":
            out_names.append(name)
            shape = tuple(alloc.tensor_shape)
            dtype = mybir.dt.np(alloc.dtype)
            out_avals.append(jax.core.ShapedArray(shape, dtype))
            zero_shapes.append((shape, dtype))
    n_params = len(in_names)
    n_outs = len(out_names)
    all_names = in_names + out_names
    if partition_name is not None:
        all_names = all_names + [partition_name]
    donate = tuple(range(n_params, n_params + n_outs))

    def _body(*args):
        operands = list(args)
        if partition_name is not None:
            operands.append(bass2jax.partition_id_tensor())
        outs = bass2jax._bass_exec_p.bind(
            *operands,
            out_avals=tuple(out_avals),
            in_names=tuple(all_names),
            out_names=tuple(out_names),
            lowering_input_output_aliases=(),
            sim_require_finite=True,
            sim_require_nnan=True,
            nc=nc,
        )
        return tuple(outs)

    devices = jax.devices()[:NCORES]
    mesh = Mesh(np.asarray(devices), ("core",))
    spec = PartitionSpec("core")
    in_specs = (spec,) * (n_params + n_outs)
    out_specs = (spec,) * n_outs
    sharded = jax.jit(
        shard_map(
            _body, mesh=mesh, in_specs=in_specs, out_specs=out_specs, check_rep=False
        ),
        donate_argnums=donate,
        keep_unused=True,
    )
    sharding = NamedSharding(mesh, spec)

    import jax.numpy as jnp

    # Donated zero output buffers are materialized ON DEVICE by this tiny
    # cached jit — uploading 100MB of host zeros through the axon tunnel
    # costs ~1s/call otherwise.
    zeros_fn = jax.jit(
        lambda: tuple(
            jnp.zeros((NCORES * s[0],) + tuple(s[1:]), d) for s, d in zero_shapes
        ),
        out_shardings=(sharding,) * n_outs,
    )
    return sharded, in_names, out_names, zeros_fn, sharding


def _weights_fingerprint_ok(args):
    saved = _cache.get("w_args")
    if saved is None:
        return False
    return all(np.array_equal(a, b) for a, b in zip(saved, args))


def kernel(src, tgt, hidden, W_ih, W_hh, b_ih, b_hh, W_fc, b_fc, **_unused):
    import jax

    if "nc" not in _cache:
        _cache["nc"] = _build_program()
        _cache["runner"] = _get_runner(_cache["nc"])
    nc = _cache["nc"]
    sharded, in_names, out_names, zeros_fn, sharding = _cache["runner"]

    w_args = (W_ih, W_hh, b_ih, b_hh, W_fc, b_fc)
    if not _weights_fingerprint_ok(w_args):
        w = _prep_weights(*w_args)
        dev = {}
        for k, v in w.items():
            g = np.ascontiguousarray(np.tile(v, (NCORES,) + (1,) * (v.ndim - 1)))
            dev[k] = jax.device_put(g, sharding)
        _cache["w_dev"] = dev
        _cache["w_args"] = tuple(np.asarray(a) for a in w_args)

    feeds = dict(_cache["w_dev"])
    feeds["xh0T"] = _prep_percall(src, hidden)

    zeros = _cache.pop("zeros_next", None)
    if zeros is None:
        zeros = zeros_fn()
    args = [feeds[nm] for nm in in_names] + list(zeros)
    outs = sharded(*args)
    _cache["zeros_next"] = zeros_fn()  # prefetch for the next call (async)

    q = dict(zip(out_names, outs))["out"]  # [8*(T+NSCL), BL, OUT] int8
    q_shards = q.addressable_shards
    q_datas = jax.device_get([sh.data for sh in q_shards])

    rows = q_datas[0].shape[0]  # T + NSCL
    full = np.empty((T, B, OUT), np.float32)
    for sh, data in zip(q_shards, q_datas):
        c = (sh.index[0].start or 0) // rows
        sbytes = np.ascontiguousarray(
            data[T:].transpose(1, 0, 2).reshape(BL, -1)[:, : T * 4]
        )
        scales = sbytes.view(np.float32)  # [BL, T]
        scal = (scales.T * (1.0 / 126.0))[:, :, None]  # [T, BL, 1]
        np.multiply(data[:T], scal, out=full[:, c * BL : (c + 1) * BL, :])
    return full


# revision 36
# speedup vs baseline: 1.0908x; 1.0394x over previous
"""GRU decoder kernel for 8 trn2 NeuronCores — batch-data-parallel, no collectives.

Algorithm (derived from the reference GruDecoder):
  x_{t+1} = y_t = h_{t+1} @ W_fc.T + b_fc, so for t>=1 the input-path matmul
  folds into the recurrence:
      gi_t = h_t @ (W_ih @ W_fc).T + (b_ih + W_ih @ b_fc)
  r/z gates use gi+gh, so those rows of the folded matrix and W_hh are summed
  host-side; the n-gate keeps gi_n / gh_n separate (r multiplies only gh_n).

Sharding: pure data-parallel over batch. Core c owns batch rows [32c, 32c+32).
  The T=256 sequential loop runs locally per core with NO collectives (the
  baseline's per-step AllGather cost ~20ms each through the axon relay).

Per-core per-step work:
  gates.T [4x1024, 32] = W_all.T-tiles @ h.T-tiles   (weight-stationary, PE)
  y.T     [32, 768]    = h.T-tiles.T @ W_fc.T        (batch-stationary, PE)
  elementwise r/z/n/h_new on [128, 256] tiles        (scalar + vector engines)
  Biases are folded into the matmuls via an extra contraction tile whose rhs
  is a ones-row tile ("ones" below).

h lives on-chip as hT [128, 8*32] bf16: partition p, col j*32+b <-> h[b, 128j+p].
The elementwise output lands directly in this layout, so no transposes at all.
y is produced batch-major [32, 768] so the host only concatenates batch slices.
"""

import os
import sys

sys.path.insert(0, "/opt/trn_rl_repo")

import numpy as np

H = 1024
OUT = 768
B = 256
T = int(os.environ.get("GRU_T", "256"))
NCORES = 8
BL = B // NCORES  # 32 batch rows per core
KH = H // 128  # 8 contraction tiles over hidden
KX = OUT // 128  # 6 contraction tiles over x (=768)

_cache = {}


def _build_program():
    import concourse.mybir as mybir
    from concourse import bacc, tile

    dt = mybir.dt
    AF = mybir.ActivationFunctionType

    nc = bacc.Bacc(num_devices=NCORES)

    wA_d = nc.dram_tensor("wA", [128, KX + 1, 3 * H], dt.bfloat16, kind="ExternalInput")
    wB_d = nc.dram_tensor("wB", [128, KH + 1, 3 * H], dt.bfloat16, kind="ExternalInput")
    wrec_d = nc.dram_tensor("wrec", [128, KH + 1, 4 * H], dt.bfloat16, kind="ExternalInput")
    wfc_d = nc.dram_tensor("wfc", [128, KH + 1, OUT], dt.bfloat16, kind="ExternalInput")
    xh0T_d = nc.dram_tensor(
        "xh0T", [128, KX + KH, BL], dt.bfloat16, kind="ExternalInput"
    )
    ones_d = nc.dram_tensor("ones", [128, BL], dt.bfloat16, kind="ExternalInput")
    # rows 0..T-1: int8 y; rows T..T+NSCL-1 hold the f32 scales bitcast to bytes
    NSCL = (BL * T * 4 + BL * OUT - 1) // (BL * OUT)
    out_d = nc.dram_tensor("out", [T + NSCL, BL, OUT], dt.int8, kind="ExternalOutput")

    with tile.TileContext(nc) as tc:
        with (
            tc.tile_pool(name="wp", bufs=1) as wp,
            tc.tile_pool(name="hp", bufs=3) as hp,
            tc.tile_pool(name="ep", bufs=1) as ep,
            tc.tile_pool(name="yp", bufs=2) as yp,
            tc.tile_pool(name="pp", bufs=1, space="PSUM") as pp,
            tc.tile_pool(name="qp", bufs=2, space="PSUM") as qp,
        ):
            wA = wp.tile([128, KX + 1, 3 * H], dt.bfloat16)
            nc.sync.dma_start(wA[:], wA_d[:])
            wB = wp.tile([128, KH + 1, 3 * H], dt.bfloat16)
            nc.sync.dma_start(wB[:], wB_d[:])
            wrec = wp.tile([128, KH + 1, 4 * H], dt.bfloat16)
            nc.sync.dma_start(wrec[:], wrec_d[:])
            wfc = wp.tile([128, KH + 1, OUT], dt.bfloat16)
            nc.sync.dma_start(wfc[:], wfc_d[:])
            ones = wp.tile([128, BL], dt.bfloat16)
            nc.sync.dma_start(ones[:], ones_d[:])
            x0T = wp.tile([128, KX, BL], dt.bfloat16)
            nc.sync.dma_start(x0T[:], xh0T_d[:, 0:KX, :])

            h = hp.tile([128, KH * BL], dt.bfloat16, tag="h")
            nc.sync.dma_start(
                h[:], xh0T_d[:, KX : KX + KH, :].rearrange("p k b -> p (k b)")
            )

            scl_all = wp.tile([BL, T], dt.float32)

            def hblk(ht, k):
                return ht[:, k * BL : (k + 1) * BL]

            def emit_gates_rec(ht):
                """Recurrent-step gates: 4 psum tiles [128, 8*32]."""
                P = {}
                for g in ("r", "z", "ni", "nh"):
                    P[g] = pp.tile([128, KH * BL], dt.float32, tag=f"P{g}", name=f"P{g}")
                for gi, g in enumerate(("r", "z", "ni", "nh")):
                    for j in range(KH):
                        o = P[g][:, j * BL : (j + 1) * BL]
                        m0 = gi * H + j * 128
                        for k in range(KH + 1):
                            nc.tensor.matmul(
                                o,
                                wrec[:, k, m0 : m0 + 128],
                                hblk(ht, k) if k < KH else ones[:],
                                start=(k == 0),
                                stop=(k == KH),
                            )
                return P

            def emit_gates_step0(ht):
                """Step 0: gi from x0 (wA: r,z,ni), gh from h0 (wB: r,z,nh)."""
                P = {}
                for g in ("r", "z", "ni", "nh"):
                    P[g] = pp.tile([128, KH * BL], dt.float32, tag=f"P{g}", name=f"P{g}")
                gidx_A = {"r": 0, "z": 1, "ni": 2}
                gidx_B = {"r": 0, "z": 1, "nh": 2}
                for g in ("r", "z", "ni", "nh"):
                    for j in range(KH):
                        o = P[g][:, j * BL : (j + 1) * BL]
                        started = False
                        if g in gidx_A:
                            m0 = gidx_A[g] * H + j * 128
                            for k in range(KX + 1):
                                nc.tensor.matmul(
                                    o,
                                    wA[:, k, m0 : m0 + 128],
                                    x0T[:, k, :] if k < KX else ones[:],
                                    start=(k == 0),
                                    stop=(k == KX and g == "ni"),
                                )
                            started = True
                        if g in gidx_B:
                            m0 = gidx_B[g] * H + j * 128
                            for k in range(KH + 1):
                                nc.tensor.matmul(
                                    o,
                                    wB[:, k, m0 : m0 + 128],
                                    hblk(ht, k) if k < KH else ones[:],
                                    start=(k == 0 and not started),
                                    stop=(k == KH),
                                )
                return P

            def emit_elem(P, ht):
                r = ep.tile([128, KH * BL], dt.float32, tag="r")
                nc.scalar.activation(r[:], P["r"][:], AF.Sigmoid)
                z = ep.tile([128, KH * BL], dt.float32, tag="z")
                nc.scalar.activation(z[:], P["z"][:], AF.Sigmoid)
                t2 = ep.tile([128, KH * BL], dt.float32, tag="t2")
                nc.vector.tensor_mul(t2[:], P["nh"][:], r[:])
                t3 = ep.tile([128, KH * BL], dt.float32, tag="t3")
                nc.vector.tensor_add(t3[:], t2[:], P["ni"][:])
                n = ep.tile([128, KH * BL], dt.float32, tag="n")
                nc.scalar.activation(n[:], t3[:], AF.Tanh)
                d = ep.tile([128, KH * BL], dt.float32, tag="d")
                nc.vector.tensor_sub(d[:], ht[:], n[:])
                zd = ep.tile([128, KH * BL], dt.float32, tag="zd")
                nc.vector.tensor_mul(zd[:], z[:], d[:])
                h_new = hp.tile([128, KH * BL], dt.bfloat16, tag="h")
                nc.vector.tensor_add(h_new[:], n[:], zd[:])
                return h_new

            def emit_y(ht, t_out):
                """y = f(ht) [32, 768], int8-quantized with a per-row scale,
                -> out_d[t_out]; the scale (row abs-max) lands in scl_all."""
                Pys = []
                for c in range(2):
                    Py = qp.tile([BL, OUT // 2], dt.float32, tag=f"Py{c}", name=f"Py{c}")
                    cc = slice(c * (OUT // 2), (c + 1) * (OUT // 2))
                    for k in range(KH + 1):
                        nc.tensor.matmul(
                            Py[:],
                            hblk(ht, k) if k < KH else ones[:],
                            wfc[:, k, cc],
                            start=(k == 0),
                            stop=(k == KH),
                        )
                    Pys.append(Py)
                m0 = ep.tile([BL, 1], dt.float32, tag="m0")
                nc.vector.tensor_reduce(
                    m0[:], Pys[0][:], mybir.AxisListType.X, mybir.AluOpType.max,
                    apply_absolute_value=True,
                )
                m1 = ep.tile([BL, 1], dt.float32, tag="m1")
                nc.vector.tensor_reduce(
                    m1[:], Pys[1][:], mybir.AxisListType.X, mybir.AluOpType.max,
                    apply_absolute_value=True,
                )
                mm = scl_all[:, t_out : t_out + 1]
                nc.vector.tensor_max(mm, m0[:], m1[:])
                rec = ep.tile([BL, 1], dt.float32, tag="rec")
                nc.vector.reciprocal(rec[:], mm)
                rec2 = ep.tile([BL, 1], dt.float32, tag="rec2")
                nc.vector.tensor_scalar_mul(rec2[:], rec[:], 126.0)
                q = yp.tile([BL, OUT], dt.int8, tag="q")
                for c in range(2):
                    cc = slice(c * (OUT // 2), (c + 1) * (OUT // 2))
                    nc.scalar.activation(
                        q[:, cc], Pys[c][:], AF.Copy, scale=rec2[:]
                    )
                nc.sync.dma_start(out_d[t_out][:], q[:])

            for t in range(T):
                if t == 0:
                    P = emit_gates_step0(h)
                else:
                    P = emit_gates_rec(h)
                    emit_y(h, t - 1)
                h = emit_elem(P, h)
            emit_y(h, T - 1)
            sbytes = scl_all[:].bitcast(dt.int8)  # [BL, 4*T]
            off = 0
            for r2 in range(NSCL):
                w = min(OUT, 4 * T - off)
                nc.sync.dma_start(out_d[T + r2][:, 0:w], sbytes[:, off : off + w])
                off += w

    nc.compile()
    return nc


def _prep_weights(W_ih, W_hh, b_ih, b_hh, W_fc, b_fc):
    """Per-core (replicated) weight arrays in lhsT tile layouts, bf16."""
    from ml_dtypes import bfloat16

    f32 = np.float32
    W_ih = np.asarray(W_ih, f32)
    W_hh = np.asarray(W_hh, f32)
    b_ih = np.asarray(b_ih, f32)
    b_hh = np.asarray(b_hh, f32)
    W_fc = np.asarray(W_fc, f32)
    b_fc = np.asarray(b_fc, f32)

    W_comb = W_ih @ W_fc  # [3H, H]
    b_comb = b_ih + W_ih @ b_fc  # [3H]

    def ktiles(mat_T, nk, m):
        # mat_T: [K, m] -> [128, nk, m]
        return np.ascontiguousarray(
            mat_T.reshape(nk, 128, m).transpose(1, 0, 2)
        )

    def with_bias(tiles, bias_row):
        # tiles [128, nk, m] + bias ktile (row 0 = bias) -> [128, nk+1, m]
        m = tiles.shape[2]
        bt = np.zeros((128, 1, m), f32)
        bt[0, 0, :] = bias_row
        return np.concatenate([tiles, bt], axis=1)

    R, Z, N = slice(0, H), slice(H, 2 * H), slice(2 * H, 3 * H)

    W_rec = np.concatenate(
        [W_comb[R] + W_hh[R], W_comb[Z] + W_hh[Z], W_comb[N], W_hh[N]], axis=0
    )  # [4H, H]
    b_rec = np.concatenate(
        [b_comb[R] + b_hh[R], b_comb[Z] + b_hh[Z], b_comb[N], b_hh[N]]
    )
    wrec = with_bias(ktiles(W_rec.T, KH, 4 * H), b_rec)

    bA = np.concatenate([b_ih[R] + b_hh[R], b_ih[Z] + b_hh[Z], b_ih[N]])
    wA = with_bias(ktiles(np.ascontiguousarray(W_ih.T), KX, 3 * H), bA)

    bB = np.zeros(3 * H, f32)
    bB[2 * H :] = b_hh[N]
    wB = with_bias(ktiles(np.ascontiguousarray(W_hh.T), KH, 3 * H), bB)

    wfc = with_bias(ktiles(np.ascontiguousarray(W_fc.T), KH, OUT), b_fc)

    ones = np.zeros((128, BL), f32)
    ones[0, :] = 1.0

    bf = bfloat16
    return {
        "wA": wA.astype(bf),
        "wB": wB.astype(bf),
        "wrec": wrec.astype(bf),
        "wfc": wfc.astype(bf),
        "ones": ones.astype(bf),
    }


def _prep_percall(src, hidden):
    """Global (concat over cores) [x0T; h0T] in one array, bf16."""
    from ml_dtypes import bfloat16

    f32 = np.float32
    x0 = np.asarray(src[0], f32)  # [B, OUT]
    h0 = np.asarray(hidden[0], f32)  # [B, H]
    xh = np.empty((NCORES, 128, KX + KH, BL), f32)
    # [c, p, k, b] = x0[32c+b, 128k+p] for k<KX, h0[32c+b, 128(k-KX)+p] after
    xh[:, :, :KX, :] = x0.reshape(NCORES, BL, KX, 128).transpose(0, 3, 2, 1)
    xh[:, :, KX:, :] = h0.reshape(NCORES, BL, KH, 128).transpose(0, 3, 2, 1)
    return xh.reshape(NCORES * 128, KX + KH, BL).astype(bfloat16)


def _get_runner(nc):
    """Cached jit over shard_map of the bass_exec custom call.

    Mirrors concourse.bass2jax.run_bass_via_pjrt's multi-core branch, but the
    jit object is built once so later calls skip retracing, and weight arrays
    can stay device-resident between calls (they are not donated).
    """
    import jax
    import concourse.mybir as mybir
    from concourse import bass2jax
    from jax.sharding import Mesh, PartitionSpec, NamedSharding
    from jax.experimental.shard_map import shard_map

    bass2jax.install_neuronx_cc_hook()
    assert nc.dbg_addr is None
    partition_name = nc.partition_id_tensor.name if nc.partition_id_tensor else None

    in_names = []
    out_names = []
    out_avals = []
    zero_shapes = []
    for alloc in nc.m.functions[0].allocations:
        if not isinstance(alloc, mybir.MemoryLocationSet):
            continue
        name = alloc.memorylocations[0].name
        if alloc.kind == "ExternalInput":
            if name != partition_name:
                in_names.append(name)
        elif alloc.kind == "ExternalOutput":
            out_names.append(name)
            shape = tuple(alloc.tensor_shape)
            dtype = mybir.dt.np(alloc.dtype)
            out_avals.append(jax.core.ShapedArray(shape, dtype))
            zero_shapes.append((shape, dtype))
    n_params = len(in_names)
    n_outs = len(out_names)
    all_names = in_names + out_names
    if partition_name is not None:
        all_names = all_names + [partition_name]
    donate = tuple(range(n_params, n_params + n_outs))

    def _body(*args):
        operands = list(args)
        if partition_name is not None:
            operands.append(bass2jax.partition_id_tensor())
        outs = bass2jax._bass_exec_p.bind(
            *operands,
            out_avals=tuple(out_avals),
            in_names=tuple(all_names),
            out_names=tuple(out_names),
            lowering_input_output_aliases=(),
            sim_require_finite=True,
            sim_require_nnan=True,
            nc=nc,
        )
        return tuple(outs)

    devices = jax.devices()[:NCORES]
    mesh = Mesh(np.asarray(devices), ("core",))
    spec = PartitionSpec("core")
    in_specs = (spec,) * (n_params + n_outs)
    out_specs = (spec,) * n_outs
    sharded = jax.jit(
        shard_map(
            _body, mesh=mesh, in_specs=in_specs, out_specs=out_specs, check_rep=False
        ),
        donate_argnums=donate,
        keep_unused=True,
    )
    sharding = NamedSharding(mesh, spec)

    import jax.numpy as jnp

    # Donated zero output buffers are materialized ON DEVICE by this tiny
    # cached jit — uploading 100MB of host zeros through the axon tunnel
    # costs ~1s/call otherwise.
    zeros_fn = jax.jit(
        lambda: tuple(
            jnp.zeros((NCORES * s[0],) + tuple(s[1:]), d) for s, d in zero_shapes
        ),
        out_shardings=(sharding,) * n_outs,
    )
    return sharded, in_names, out_names, zeros_fn, sharding


def _weights_fingerprint_ok(args):
    saved = _cache.get("w_args")
    if saved is None:
        return False
    return all(np.array_equal(a, b) for a, b in zip(saved, args))


def kernel(src, tgt, hidden, W_ih, W_hh, b_ih, b_hh, W_fc, b_fc, **_unused):
    import jax

    if "nc" not in _cache:
        _cache["nc"] = _build_program()
        _cache["runner"] = _get_runner(_cache["nc"])
    nc = _cache["nc"]
    sharded, in_names, out_names, zeros_fn, sharding = _cache["runner"]

    w_args = (W_ih, W_hh, b_ih, b_hh, W_fc, b_fc)
    if not _weights_fingerprint_ok(w_args):
        w = _prep_weights(*w_args)
        dev = {}
        for k, v in w.items():
            g = np.ascontiguousarray(np.tile(v, (NCORES,) + (1,) * (v.ndim - 1)))
            dev[k] = jax.device_put(g, sharding)
        _cache["w_dev"] = dev
        _cache["w_args"] = tuple(np.asarray(a) for a in w_args)

    feeds = dict(_cache["w_dev"])
    feeds["xh0T"] = _prep_percall(src, hidden)

    zeros = _cache.pop("zeros_next", None)
    if zeros is None:
        zeros = zeros_fn()
    args = [feeds[nm] for nm in in_names] + list(zeros)
    outs = sharded(*args)

    q = dict(zip(out_names, outs))["out"]  # [8*(T+NSCL), BL, OUT] int8
    q_shards = q.addressable_shards
    bufs = [sh.data for sh in q_shards]
    # queue all D2H copies up front (they start as soon as execution lands),
    # then dequantize shard i while shards i+1.. are still in flight
    for b in bufs:
        try:
            b.copy_to_host_async()
        except Exception:
            pass
    rows = bufs[0].shape[0]  # T + NSCL
    full = np.empty((T, B, OUT), np.float32)
    for sh, b in zip(q_shards, bufs):
        data = np.asarray(b)
        c = (sh.index[0].start or 0) // rows
        sbytes = np.ascontiguousarray(
            data[T:].transpose(1, 0, 2).reshape(BL, -1)[:, : T * 4]
        )
        scales = sbytes.view(np.float32)  # [BL, T]
        scal = (scales.T * (1.0 / 126.0))[:, :, None]  # [T, BL, 1]
        np.multiply(data[:T], scal, out=full[:, c * BL : (c + 1) * BL, :])
    _cache["zeros_next"] = zeros_fn()  # prefetch for the next call (async)
    return full
